# revision 6
# baseline (speedup 1.0000x reference)
"""Trainium2 Bass kernel for nn_CoefficientDecoder.

reference computation (all f32):
    h = relu(x @ W1.T + b1)         x:[B,256] -> h:[B,64]
    h = h @ Wd3.T + bd3             [B,64]
    h = h @ Wd2.T + bd2             [B,64]
    h = h @ Wd1.T + bd1             [B,64]
    z = h @ W2.T + b2               [B,512]
    out = z @ bases                 bases:[512,4096] -> out:[B,4096]

Everything after the relu is linear, so it all folds host-side into one
[65, 4096] matrix:

    W2eff = W2 @ Wd1 @ Wd2 @ Wd3                       [512, 64]
    b2eff = b2 + (bd3 @ Wd2.T @ Wd1.T + bd2 @ Wd1.T + bd1) @ W2.T
    Beff  = [[W2eff.T @ bases], [b2eff @ bases]]       [65, 4096]
    out   = [h, 1] @ Beff

The constant-1 column of h comes for free by augmenting layer 1 with a
65th output unit (zero weights, bias 1, relu(1)=1).  This removes the 8 MB
bases load and cuts the device flops ~8x: the kernel is then HBM-bound on
the 16 MB/core output, so everything runs in f16 (x, W1, Beff inputs and
the output), cutting per-core traffic from ~25 MB to ~9 MB.  f16 keeps
~5e-4 norm rel-err, far under the 2e-2 gate, and f16 PE operands need no
fp32r rounding copies.

Strategy: pure data-parallel over the batch dim across 8 NeuronCores
(B=8192 -> 1024 rows/core); constants replicated per core.

Per-core schedule (all matmuls f16 x f16 -> f32 PSUM):
    loads: Beff [65,4096] on the SP HWDGE ring; W1/b1/xT on the ACT ring.
    MLP:   hT[65,1024] = relu(W1aug @ xT + b1aug), 2 psum banks, ACT bias+relu
    GEMM:  for mm in 8:  stationary hT[:, mm*128:+128]
              for s in 8: psum[128,512] = hT_mm.T @ Beff[:, s*512:+512]
              PSUM->SBUF f16 copies alternate DVE/ACT into ob[128,4096]
           one 1 MB dma_start per mm block on the SP ring (8 KB/row,
           descriptor-efficient), double-buffered ob.

`repeat` wraps the body in a hardware For_i loop - used only for timing
(amortizes the ~100 ms axon dispatch overhead).
"""

import numpy as np

import concourse.bass as bass
import concourse.tile as tile
from concourse import bacc, mybir
from concourse.bass import ts
from concourse.bass_utils import run_bass_kernel_spmd

N_CORES = 8
B, IN_F, HID, NB, SEQ = 8192, 256, 64, 512, 4096
B_LOC = B // N_CORES            # 1024 batch rows per core
HID1 = HID + 1                  # hidden + constant-1 unit

F32 = mybir.dt.float32
F16 = mybir.dt.float16

GEMM_MODE = "f16"
OUT_MODE = "f16"

KC = IN_F // 128                # 2 k-chunks for layer 1
NJ = B_LOC // 512               # 2 batch chunks for the MLP moving dim
MM = B_LOC // 128               # 8 batch sub-chunks for the final GEMM
SC = SEQ // 512                 # 8 seq chunks

_CACHE = {}


def _build(gemm_mode: str = GEMM_MODE, out_mode: str = OUT_MODE, repeat: int = 1):
    assert gemm_mode == "f16" and out_mode == "f16"

    nc = bacc.Bacc(
        "TRN2",
        target_bir_lowering=False,
        debug=False,
        enable_asserts=False,
        num_devices=N_CORES,
    )

    xT_d = nc.declare_dram_parameter("xT", [IN_F, B_LOC], F16, isOutput=False)
    wb_d = nc.declare_dram_parameter("wb", [128, KC * HID1], F16, isOutput=False)
    b1_d = nc.declare_dram_parameter("b1", [HID1, 1], F32, isOutput=False)
    beff_d = nc.declare_dram_parameter("beff", [HID1, SEQ], F16, isOutput=False)
    out_d = nc.declare_dram_parameter("out", [B_LOC, SEQ], F16, isOutput=True)

    relu = mybir.ActivationFunctionType.Relu
    copyf = mybir.ActivationFunctionType.Copy

    with tile.TileContext(nc) as tc:
        with (
            tc.tile_pool(name="const", bufs=2) as constp,
            tc.tile_pool(name="xin", bufs=2) as xp,
            tc.tile_pool(name="hbuf", bufs=2) as hp,
            tc.tile_pool(name="outsb", bufs=3) as outsbp,
            tc.tile_pool(name="ps", bufs=8, space="PSUM") as psp,
        ):
            def body():
                # all loads on the ACT ring (MLP-critical first); out stores
                # get the SP ring to themselves + every other one on ACT
                wb = constp.tile([128, KC * HID1], F16, tag="wb")
                nc.scalar.dma_start(wb[:], wb_d[:])
                b1 = constp.tile([HID1, 1], F32, tag="b1")
                nc.scalar.dma_start(b1[:], b1_d[:])
                xT = xp.tile([128, KC, B_LOC], F16, tag="xT")
                xT_pkn = xT_d.rearrange("(k p) n -> p k n", p=128)
                nc.scalar.dma_start(xT[:], xT_pkn[:])
                beff = constp.tile([HID1, SEQ], F16, tag="beff")
                nc.scalar.dma_start(beff[:], beff_d[:])

                # ---- MLP: hT [65, B_LOC] = relu(W1aug @ xT + b1aug) ----
                h1 = hp.tile([HID1, B_LOC], F16, tag="h1")
                for j in range(NJ):
                    ps = psp.tile([HID1, 512], F32, tag="ps")
                    for k in range(KC):
                        nc.tensor.matmul(
                            ps[:],
                            wb[:, k * HID1 : (k + 1) * HID1],
                            xT[:, k, ts(j, 512)],
                            start=(k == 0),
                            stop=(k == KC - 1),
                        )
                    nc.scalar.activation(h1[:, ts(j, 512)], ps[:], relu, bias=b1[:])

                # ---- final GEMM: out[mm*128:+128, :] = h1_mm.T @ Beff ----
                for mm_i in range(MM):
                    ob = outsbp.tile([128, SEQ], F16, tag="ob")
                    for s in range(SC):
                        op = psp.tile([128, 512], F32, tag="ps")
                        nc.tensor.matmul(
                            op[:],
                            h1[:, ts(mm_i, 128)],
                            beff[:, ts(s, 512)],
                            start=True,
                            stop=True,
                        )
                        if s % 2 == 0:
                            nc.vector.tensor_copy(ob[:, ts(s, 512)], op[:])
                        else:
                            nc.scalar.activation(ob[:, ts(s, 512)], op[:], copyf)
                    eng = nc.sync if mm_i % 2 == 0 else nc.scalar
                    eng.dma_start(out_d[ts(mm_i, 128), :], ob[:])

            if repeat == 1:
                body()
            else:
                with tc.For_i(0, repeat, 1):
                    body()

    nc.compile()
    return nc


def _get_nc(gemm_mode: str = GEMM_MODE, out_mode: str = OUT_MODE, repeat: int = 1):
    key = (gemm_mode, out_mode, repeat)
    if key not in _CACHE:
        _CACHE[key] = _build(gemm_mode, out_mode, repeat)
    return _CACHE[key]


def _pack_consts(W1, b1, Wd1, bd1, Wd2, bd2, Wd3, bd3, W2, b2, bases):
    W1 = W1.astype(np.float64); b1 = b1.astype(np.float64)
    W2 = W2.astype(np.float64); b2 = b2.astype(np.float64)
    Wd1 = Wd1.astype(np.float64); Wd2 = Wd2.astype(np.float64)
    Wd3 = Wd3.astype(np.float64)
    bases = bases.astype(np.float64)

    W2eff = W2 @ Wd1 @ Wd2 @ Wd3                      # [512, 64]
    b2eff = b2 + (bd3 @ Wd2.T @ Wd1.T + bd2 @ Wd1.T + bd1) @ W2.T
    beff = np.empty((HID1, SEQ), np.float64)
    beff[:HID] = W2eff.T @ bases
    beff[HID] = b2eff @ bases

    # layer-1 weights augmented with the constant-1 unit, packed as the
    # two [128, HID1] stationary k-chunks side by side
    wb = np.zeros((128, KC * HID1), np.float16)
    W1T = W1.T                                        # [256, 64]
    for k in range(KC):
        wb[:, k * HID1 : k * HID1 + HID] = W1T[k * 128 : (k + 1) * 128]
    b1a = np.zeros((HID1, 1), np.float32)
    b1a[:HID, 0] = b1
    b1a[HID, 0] = 1.0
    return {
        "wb": wb,
        "b1": b1a,
        "beff": beff.astype(np.float16),
    }


def _in_maps(x, W1, b1, Wd1, bd1, Wd2, bd2, Wd3, bd3, W2, b2, bases,
             gemm_mode: str = GEMM_MODE):
    common = _pack_consts(W1, b1, Wd1, bd1, Wd2, bd2, Wd3, bd3, W2, b2, bases)
    maps = []
    for i in range(N_CORES):
        m = dict(common)
        m["xT"] = np.ascontiguousarray(
            x[i * B_LOC : (i + 1) * B_LOC].T.astype(np.float16))
        maps.append(m)
    return maps


def run(inputs: dict, gemm_mode: str = GEMM_MODE, out_mode: str = OUT_MODE,
        repeat: int = 1, **run_kwargs):
    """Shard, execute on 8 cores, gather. Returns (out, BassKernelResults)."""
    nc = _get_nc(gemm_mode, out_mode, repeat)
    in_maps = _in_maps(**{k: np.asarray(v) for k, v in inputs.items()},
                       gemm_mode=gemm_mode)
    res = run_bass_kernel_spmd(nc, in_maps, list(range(N_CORES)), **run_kwargs)
    shards = [np.asarray(res.results[i]["out"], dtype=np.float32)
              for i in range(N_CORES)]
    out = np.concatenate(shards, axis=0)
    return out, res


def kernel(**inputs) -> np.ndarray:
    out, _ = run(inputs)
    return out


# revision 10
# speedup vs baseline: 1.0741x; 1.0741x over previous
"""Trainium2 Bass kernel for nn_CoefficientDecoder.

reference computation (all f32):
    h = relu(x @ W1.T + b1)         x:[B,256] -> h:[B,64]
    h = h @ Wd3.T + bd3             [B,64]
    h = h @ Wd2.T + bd2             [B,64]
    h = h @ Wd1.T + bd1             [B,64]
    z = h @ W2.T + b2               [B,512]
    out = z @ bases                 bases:[512,4096] -> out:[B,4096]

Everything after the relu is linear, so it all folds host-side into one
[65, 4096] matrix:

    W2eff = W2 @ Wd1 @ Wd2 @ Wd3                       [512, 64]
    b2eff = b2 + (bd3 @ Wd2.T @ Wd1.T + bd2 @ Wd1.T + bd1) @ W2.T
    Beff  = [[W2eff.T @ bases], [b2eff @ bases]]       [65, 4096]
    out   = [h, 1] @ Beff

The constant-1 column of h comes for free by augmenting layer 1 with a
65th output unit (zero weights, bias 1, relu(1)=1).  This removes the 8 MB
bases load and cuts the device flops ~8x: the kernel is then HBM-bound on
the 16 MB/core output, so everything runs in f16 (x, W1, Beff inputs and
the output), cutting per-core traffic from ~25 MB to ~9 MB.  f16 keeps
~5e-4 norm rel-err, far under the 2e-2 gate, and f16 PE operands need no
fp32r rounding copies.

Strategy: pure data-parallel over the batch dim across 8 NeuronCores
(B=8192 -> 1024 rows/core); constants replicated per core.

Per-core schedule (all matmuls f16 x f16 -> f32 PSUM):
    loads: Beff [65,4096] on the SP HWDGE ring; W1/b1/xT on the ACT ring.
    MLP:   hT[65,1024] = relu(W1aug @ xT + b1aug), 2 psum banks, ACT bias+relu
    GEMM:  for mm in 8:  stationary hT[:, mm*128:+128]
              for s in 8: psum[128,512] = hT_mm.T @ Beff[:, s*512:+512]
              PSUM->SBUF f16 copies alternate DVE/ACT into ob[128,4096]
           one 1 MB dma_start per mm block on the SP ring (8 KB/row,
           descriptor-efficient), double-buffered ob.

`repeat` wraps the body in a hardware For_i loop - used only for timing
(amortizes the ~100 ms axon dispatch overhead).
"""

import os

import numpy as np

import concourse.bass as bass
import concourse.tile as tile
from concourse import bacc, mybir
from concourse.bass import ts
from concourse.bass_utils import run_bass_kernel_spmd

N_CORES = 8
B, IN_F, HID, NB, SEQ = 8192, 256, 64, 512, 4096
B_LOC = B // N_CORES            # 1024 batch rows per core
HID1 = HID + 1                  # hidden + constant-1 unit

F32 = mybir.dt.float32
F16 = mybir.dt.float16

GEMM_MODE = "f16"
OUT_MODE = "f16"

KC = IN_F // 128                # 2 k-chunks for layer 1
NJ = B_LOC // 512               # 2 batch chunks for the MLP moving dim
MM = B_LOC // 128               # 8 batch sub-chunks for the final GEMM
SC = SEQ // 512                 # 8 seq chunks

_CACHE = {}

# ablation knob (timing experiments only): full | nostore | dmaonly
VAR = os.environ.get("KVAR", "full")


def _build(gemm_mode: str = GEMM_MODE, out_mode: str = OUT_MODE, repeat: int = 1):
    assert gemm_mode == "f16" and out_mode == "f16"

    nc = bacc.Bacc(
        "TRN2",
        target_bir_lowering=False,
        debug=False,
        enable_asserts=False,
        num_devices=N_CORES,
    )

    xT_d = nc.declare_dram_parameter("xT", [IN_F, B_LOC], F16, isOutput=False)
    wb_d = nc.declare_dram_parameter("wb", [128, KC * HID1], F16, isOutput=False)
    b1_d = nc.declare_dram_parameter("b1", [HID1, 1], F32, isOutput=False)
    beff_d = nc.declare_dram_parameter("beff", [HID1, SEQ], F16, isOutput=False)
    out_d = nc.declare_dram_parameter("out", [B_LOC, SEQ], F16, isOutput=True)

    relu = mybir.ActivationFunctionType.Relu
    copyf = mybir.ActivationFunctionType.Copy

    with tile.TileContext(nc) as tc:
        with (
            tc.tile_pool(name="const", bufs=2) as constp,
            tc.tile_pool(name="xin", bufs=2) as xp,
            tc.tile_pool(name="hbuf", bufs=2) as hp,
            tc.tile_pool(name="outsb", bufs=3) as outsbp,
            tc.tile_pool(name="ps", bufs=8, space="PSUM") as psp,
        ):
            obx = None
            if VAR == "dmaonly":
                obx = outsbp.tile([128, SEQ], F16, tag="obx")
                nc.vector.memset(obx[:], 0.25)

            def body():
                # all loads on the ACT ring (MLP-critical first); out stores
                # get the SP ring to themselves + every other one on ACT
                wb = constp.tile([128, KC * HID1], F16, tag="wb")
                nc.scalar.dma_start(wb[:], wb_d[:])
                b1 = constp.tile([HID1, 1], F32, tag="b1")
                nc.scalar.dma_start(b1[:], b1_d[:])
                xT = xp.tile([128, KC, B_LOC], F16, tag="xT")
                xT_pkn = xT_d.rearrange("(k p) n -> p k n", p=128)
                nc.scalar.dma_start(xT[:], xT_pkn[:])
                beff = constp.tile([HID1, SEQ], F16, tag="beff")
                nc.scalar.dma_start(beff[:], beff_d[:])

                if VAR == "dmaonly":
                    for mm_i in range(MM):
                        eng = nc.sync if mm_i % 2 == 0 else nc.scalar
                        eng.dma_start(out_d[ts(mm_i, 128), :], obx[:])
                    return

                # ---- MLP: hT [65, B_LOC] = relu(W1aug @ xT + b1aug) ----
                h1 = hp.tile([HID1, B_LOC], F16, tag="h1")
                for j in range(NJ):
                    ps = psp.tile([HID1, 512], F32, tag="ps")
                    for k in range(KC):
                        nc.tensor.matmul(
                            ps[:],
                            wb[:, k * HID1 : (k + 1) * HID1],
                            xT[:, k, ts(j, 512)],
                            start=(k == 0),
                            stop=(k == KC - 1),
                        )
                    nc.scalar.activation(h1[:, ts(j, 512)], ps[:], relu, bias=b1[:])

                # ---- final GEMM: out[mm*128:+128, :] = h1_mm.T @ Beff ----
                for mm_i in range(MM):
                    ob = outsbp.tile([128, SEQ], F16, tag="ob")
                    for s in range(SC):
                        op = psp.tile([128, 512], F32, tag="ps")
                        nc.tensor.matmul(
                            op[:],
                            h1[:, ts(mm_i, 128)],
                            beff[:, ts(s, 512)],
                            start=True,
                            stop=True,
                        )
                        if s % 2 == 0:
                            nc.vector.tensor_copy(ob[:, ts(s, 512)], op[:])
                        else:
                            nc.scalar.activation(ob[:, ts(s, 512)], op[:], copyf)
                    if VAR != "nostore":
                        eng = nc.sync if mm_i % 2 == 0 else nc.scalar
                        eng.dma_start(out_d[ts(mm_i, 128), :], ob[:])

            if repeat == 1:
                body()
            else:
                with tc.For_i(0, repeat, 1):
                    body()

    nc.compile()
    return nc


def _get_nc(gemm_mode: str = GEMM_MODE, out_mode: str = OUT_MODE, repeat: int = 1):
    key = (gemm_mode, out_mode, repeat)
    if key not in _CACHE:
        _CACHE[key] = _build(gemm_mode, out_mode, repeat)
    return _CACHE[key]


def _pack_consts(W1, b1, Wd1, bd1, Wd2, bd2, Wd3, bd3, W2, b2, bases):
    W1 = W1.astype(np.float64); b1 = b1.astype(np.float64)
    W2 = W2.astype(np.float64); b2 = b2.astype(np.float64)
    Wd1 = Wd1.astype(np.float64); Wd2 = Wd2.astype(np.float64)
    Wd3 = Wd3.astype(np.float64)
    bases = bases.astype(np.float64)

    W2eff = W2 @ Wd1 @ Wd2 @ Wd3                      # [512, 64]
    b2eff = b2 + (bd3 @ Wd2.T @ Wd1.T + bd2 @ Wd1.T + bd1) @ W2.T
    beff = np.empty((HID1, SEQ), np.float64)
    beff[:HID] = W2eff.T @ bases
    beff[HID] = b2eff @ bases

    # layer-1 weights augmented with the constant-1 unit, packed as the
    # two [128, HID1] stationary k-chunks side by side
    wb = np.zeros((128, KC * HID1), np.float16)
    W1T = W1.T                                        # [256, 64]
    for k in range(KC):
        wb[:, k * HID1 : k * HID1 + HID] = W1T[k * 128 : (k + 1) * 128]
    b1a = np.zeros((HID1, 1), np.float32)
    b1a[:HID, 0] = b1
    b1a[HID, 0] = 1.0
    return {
        "wb": wb,
        "b1": b1a,
        "beff": beff.astype(np.float16),
    }


def _in_maps(x, W1, b1, Wd1, bd1, Wd2, bd2, Wd3, bd3, W2, b2, bases,
             gemm_mode: str = GEMM_MODE):
    common = _pack_consts(W1, b1, Wd1, bd1, Wd2, bd2, Wd3, bd3, W2, b2, bases)
    maps = []
    for i in range(N_CORES):
        m = dict(common)
        m["xT"] = np.ascontiguousarray(
            x[i * B_LOC : (i + 1) * B_LOC].T.astype(np.float16))
        maps.append(m)
    return maps


def run(inputs: dict, gemm_mode: str = GEMM_MODE, out_mode: str = OUT_MODE,
        repeat: int = 1, **run_kwargs):
    """Shard, execute on 8 cores, gather. Returns (out, BassKernelResults)."""
    nc = _get_nc(gemm_mode, out_mode, repeat)
    in_maps = _in_maps(**{k: np.asarray(v) for k, v in inputs.items()},
                       gemm_mode=gemm_mode)
    res = run_bass_kernel_spmd(nc, in_maps, list(range(N_CORES)), **run_kwargs)
    shards = [np.asarray(res.results[i]["out"], dtype=np.float32)
              for i in range(N_CORES)]
    out = np.concatenate(shards, axis=0)
    return out, res


def kernel(**inputs) -> np.ndarray:
    out, _ = run(inputs)
    return out
